# revision 1
# baseline (speedup 1.0000x reference)
"""Trainium2 Bass kernel for nn_DAttention:
out[b,c,d,h,w] = x[b,c,d,h,w] * mean_{c,h,w}(x[b,:,d,:,:]).

Sharding: pure data parallel over batch B=8 -> one batch per NeuronCore
(x[b] is a contiguous zero-copy slice). Per core, loop over the 32 d-slices
(2 MiB each): load x[b,:,d,:,:] into SBUF, reduce to the scalar mean,
multiply in SBUF, store. Single pass over HBM: 64 MiB read + 64 MiB
written per core — the memory-roofline minimum.

SBUF layout per d-slice: tile [128, 4096] with partition p = c*4 + hg
(H split into 4 groups of 32), free = (h%32)*128 + w. Each partition row
is one contiguous 16 KiB DRAM run -> line-rate DMA packets (~610 ns/16 KiB
per SDMA engine). Loads issue on the SP HWDGE ring, stores on the ACT ring.

Engine schedule — exactly one big op per engine per iteration (a second
large DVE op per iteration stalls streaming via the per-op pipe DRAIN):
  ACT: two activation-Copies (halves of xt) into a dead PSUM scratch with
       accum_out -> the per-partition column sums (no SBUF write traffic)
  PE : two accumulated matmuls against a constant 128x128 matrix of
       1/524288 (exact power of two) -> cross-partition sum + broadcast
       of the mean to all partitions in one PSUM [128,1]
  ACT: tiny copy of the mean PSUM->SBUF
  DVE: single tensor_scalar multiply (2x fp32 mode), the only big DVE op
  ACT: store DMA issue

Measured: ~336 us/core on trn2 — equal to the pure load+store DMA floor
for this traffic (128 MiB through 16 SDMA engines at line rate); compute
is fully hidden behind the DMA stream.
"""
import numpy as np

import concourse.bacc as bacc
import concourse.tile as tile
import concourse.mybir as mybir
from concourse.bass_utils import run_bass_kernel_spmd

B, C, D, H, W = 8, 32, 32, 128, 128
HG, HL = 4, 32          # H split: partition dim = C*HG = 128
P = C * HG              # 128 partitions
F = HL * W              # 4096 free elements per partition
N_RED = C * H * W       # 524288 = 2**19 elements reduced per (b, d)
RECIP = 1.0 / N_RED     # exact in fp32

_NC = None


def _build_nc(xin_bufs=8, out_bufs=3):
    nc = bacc.Bacc("TRN2", target_bir_lowering=False, debug=False)
    x5 = nc.dram_tensor("x", [C, D, HG, HL, W], mybir.dt.float32, kind="ExternalInput")
    o5 = nc.dram_tensor("out", [C, D, HG, HL, W], mybir.dt.float32, kind="ExternalOutput")
    half = F // 2
    with tile.TileContext(nc) as tc:
        with (
            tc.tile_pool(name="xin", bufs=xin_bufs) as xpool,
            tc.tile_pool(name="oout", bufs=out_bufs) as opool,
            tc.tile_pool(name="small", bufs=6) as spool,
            tc.tile_pool(name="psum", bufs=2, space="PSUM") as ppool,
            tc.tile_pool(name="psc", bufs=1, space="PSUM") as scpool,
            tc.tile_pool(name="const", bufs=1) as cpool,
        ):
            recip = cpool.tile([P, P], mybir.dt.float32)
            nc.gpsimd.memset(recip[:], RECIP)
            for d in range(D):
                xt = xpool.tile([P, F], mybir.dt.float32, tag="xt")
                nc.sync.dma_start(xt[:], x5[:, d])
                csa = spool.tile([P, 1], mybir.dt.float32, tag="csa")
                csb = spool.tile([P, 1], mybir.dt.float32, tag="csb")
                scratch = scpool.tile([P, half], mybir.dt.float32, tag="sc")
                nc.scalar.activation(
                    scratch[:], xt[:, :half],
                    mybir.ActivationFunctionType.Copy, accum_out=csa[:],
                )
                nc.scalar.activation(
                    scratch[:], xt[:, half:],
                    mybir.ActivationFunctionType.Copy, accum_out=csb[:],
                )
                dv = ppool.tile([P, 1], mybir.dt.float32, tag="dv")
                nc.tensor.matmul(dv[:], recip[:], csa[:], start=True, stop=False)
                nc.tensor.matmul(dv[:], recip[:], csb[:], start=False, stop=True)
                dvs = spool.tile([P, 1], mybir.dt.float32, tag="dvs")
                nc.scalar.copy(dvs[:], dv[:])
                ot = opool.tile([P, F], mybir.dt.float32, tag="ot")
                nc.vector.tensor_scalar_mul(ot[:], xt[:], dvs[:])
                nc.scalar.dma_start(o5[:, d], ot[:])
    nc.compile()
    return nc


def _get_nc():
    global _NC
    if _NC is None:
        _NC = _build_nc()
    return _NC


def run(x: np.ndarray, trace: bool = False, tmpdir: str | None = None):
    """Run on 8 NeuronCores; returns (out, BassKernelResults)."""
    x = np.asarray(x)
    assert x.shape == (B, C, D, H, W), x.shape
    x = x.astype(np.float32, copy=False)
    nc = _get_nc()
    in_maps = [
        {"x": np.ascontiguousarray(x[b]).reshape(C, D, HG, HL, W)} for b in range(B)
    ]
    res = run_bass_kernel_spmd(
        nc, in_maps, core_ids=list(range(B)), trace=trace, tmpdir=tmpdir
    )
    out = np.stack([r["out"].reshape(C, D, H, W) for r in res.results])
    return out, res


def kernel(x: np.ndarray) -> np.ndarray:
    out, _ = run(x)
    return out



# revision 7
# speedup vs baseline: 1.1840x; 1.1840x over previous
"""Trainium2 Bass kernel for nn_DAttention:
out[b,c,d,h,w] = x[b,c,d,h,w] * mean_{c,h,w}(x[b,:,d,:,:]).

Sharding: weighted data parallel over the 256 (b,d) slices (2 MiB each).
The 8 NeuronCores share ~2.8 TB/s of chip HBM bandwidth, but the HBM/NOC
arbitration is statically unfair: under full 8-core contention, jax
devices 0/1 (physical nc4/nc5) sustain only ~250-280 GB/s while devices
3/4/5/7 get the full per-core ~430 GB/s DMA line rate (measured from
all-core NTFF profiles). Uniform B-sharding therefore leaves the starved
cores running ~150 us past the rest — and max-core time is what counts.

Fix: assign each core a slice count proportional to its measured
contended bandwidth. The host packs each core's slices contiguously
(x transposed to [B,D,C,H,W] -> 256 x 2 MiB slices), so every DMA is a
fully sequential HBM stream. One SPMD NEFF runs on all cores; the
per-core count is derived on-device from partition_id() (a register
TensorLoad of an arbitrary input tensor dies on the axon/PJRT path, but
the partition-id load is plumbed specially and works), and the unequal
trip counts are realized with predicated DMAs (cond= skips the transfer
but still bumps the semaphore). Real slices sit at the TAIL of the
S_MAX iteration range: the skipped iterations' descriptors drain
instantly at the head of the queue, so their garbage compute overlaps
the first real loads instead of serializing after the last real store.

Per-slice engine schedule (one big op per engine per iteration):
  ACT: two activation-Copies (halves) into a dead PSUM scratch with
       accum_out -> per-partition column sums (no SBUF write traffic)
  PE : two accumulated matmuls against a constant 128x128 matrix of
       1/524288 -> cross-partition sum + broadcast of the mean
  ACT: tiny copy of the mean PSUM->SBUF
  DVE: single tensor_scalar multiply (2x fp32 mode)
  ACT: store DMA issue (predicated)
"""
import numpy as np

import concourse.bacc as bacc
import concourse.tile as tile
import concourse.mybir as mybir
from concourse.bass_utils import run_bass_kernel_spmd

B, C, D, H, W = 8, 32, 32, 128, 128
P = 128                 # SBUF partitions
F = 4096                # free elements per partition; P*F = one (b,d) slice
N_RED = C * H * W       # 524288 = 2**19 elements reduced per (b, d)
RECIP = 1.0 / N_RED     # exact in fp32
NSLICES = B * D         # 256
S_MAX = 40              # compiled loop bound; per-core real count <= S_MAX

# Slices per jax device, proportional to measured contended HBM bandwidth
# (device order 0..7 = physical nc 4,5,6,7,2,3,0,1). Sum must be 256.
COUNTS = [22, 24, 31, 37, 37, 37, 31, 37]
assert sum(COUNTS) == NSLICES and max(COUNTS) <= S_MAX

_NC = None


def _build_nc(xin_bufs=8, out_bufs=3):
    nc = bacc.Bacc("TRN2", target_bir_lowering=False, debug=False)
    x5 = nc.dram_tensor("x", [S_MAX, P, F], mybir.dt.float32, kind="ExternalInput")
    o5 = nc.dram_tensor("out", [S_MAX, P, F], mybir.dt.float32, kind="ExternalOutput")
    half = F // 2

    def skip_threshold(eng):
        # t = S_MAX - COUNTS[pid]; iteration s runs iff s >= t
        pid = eng.partition_id()
        t = None
        for i in range(8):
            term = (pid == i) * (S_MAX - COUNTS[i])
            t = term if t is None else t + term
        return eng.snap(t, min_val=0, max_val=S_MAX)

    with tile.TileContext(nc) as tc:
        with (
            tc.tile_pool(name="xin", bufs=xin_bufs) as xpool,
            tc.tile_pool(name="oout", bufs=out_bufs) as opool,
            tc.tile_pool(name="small", bufs=6) as spool,
            tc.tile_pool(name="psum", bufs=2, space="PSUM") as ppool,
            tc.tile_pool(name="psc", bufs=1, space="PSUM") as scpool,
            tc.tile_pool(name="const", bufs=1) as cpool,
        ):
            recip = cpool.tile([P, P], mybir.dt.float32)
            nc.gpsimd.memset(recip[:], RECIP)
            # Per-core skip threshold, computed once per DMA-issuing engine.
            t_sync = skip_threshold(nc.sync)
            t_scl = skip_threshold(nc.scalar)
            for s in range(S_MAX):
                # real slices occupy s in [S_MAX - n, S_MAX); skip the head
                xt = xpool.tile([P, F], mybir.dt.float32, tag="xt")
                nc.sync.dma_start(xt[:], x5[s], cond=(t_sync <= s))
                csa = spool.tile([P, 1], mybir.dt.float32, tag="csa")
                csb = spool.tile([P, 1], mybir.dt.float32, tag="csb")
                scratch = scpool.tile([P, half], mybir.dt.float32, tag="sc")
                nc.scalar.activation(
                    scratch[:], xt[:, :half],
                    mybir.ActivationFunctionType.Copy, accum_out=csa[:],
                )
                nc.scalar.activation(
                    scratch[:], xt[:, half:],
                    mybir.ActivationFunctionType.Copy, accum_out=csb[:],
                )
                dv = ppool.tile([P, 1], mybir.dt.float32, tag="dv")
                nc.tensor.matmul(dv[:], recip[:], csa[:], start=True, stop=False)
                nc.tensor.matmul(dv[:], recip[:], csb[:], start=False, stop=True)
                dvs = spool.tile([P, 1], mybir.dt.float32, tag="dvs")
                nc.scalar.copy(dvs[:], dv[:])
                ot = opool.tile([P, F], mybir.dt.float32, tag="ot")
                nc.vector.tensor_scalar_mul(ot[:], xt[:], dvs[:])
                nc.scalar.dma_start(o5[s], ot[:], cond=(t_scl <= s))
    nc.compile()
    return nc


def _get_nc():
    global _NC
    if _NC is None:
        _NC = _build_nc()
    return _NC


def run(x: np.ndarray, trace: bool = False, tmpdir: str | None = None):
    """Run on 8 NeuronCores; returns (out, BassKernelResults)."""
    x = np.asarray(x)
    assert x.shape == (B, C, D, H, W), x.shape
    x = x.astype(np.float32, copy=False)
    nc = _get_nc()
    # [B,C,D,H,W] -> [B,D,C,H,W] -> 256 slices of [C*H*W]; slice (b,d) is
    # contiguous so each core's shard is a pure sequential HBM stream.
    xt = np.ascontiguousarray(x.transpose(0, 2, 1, 3, 4)).reshape(NSLICES, P, F)
    offs = np.concatenate([[0], np.cumsum(COUNTS)])
    in_maps = []
    for i in range(8):
        n = COUNTS[i]
        xp = np.empty((S_MAX, P, F), dtype=np.float32)
        xp[S_MAX - n:] = xt[offs[i]:offs[i + 1]]
        in_maps.append({"x": xp})
    res = run_bass_kernel_spmd(
        nc, in_maps, core_ids=list(range(8)), trace=trace, tmpdir=tmpdir
    )
    ot = np.empty((NSLICES, P, F), dtype=np.float32)
    for i in range(8):
        n = COUNTS[i]
        ot[offs[i]:offs[i + 1]] = res.results[i]["out"][S_MAX - n:]
    out = ot.reshape(B, D, C, H, W).transpose(0, 2, 1, 3, 4)
    return np.ascontiguousarray(out), res


def kernel(x: np.ndarray) -> np.ndarray:
    out, _ = run(x)
    return out
